# revision 7
# baseline (speedup 1.0000x reference)
"""Causal attention head on 8 TRN2 NeuronCores.

reference: out = softmax(causal((x @ wqk) @ x.T)) @ x @ wov
  x: [4096, 1024] f32, wqk/wov: [1024, 1024] f32.

Sharding: sequence-parallel on query rows with stride-8 interleave -- core m
owns global rows {m, m+8, m+16, ...} (512 rows). This balances the causal
triangle perfectly across cores AND keeps the SPMD graph identical on every
core: the causal mask depends on the core only through its input data
(a host-prepared [128, 1024] additive mask), never through the graph.

Per-core layout: 512 local rows = 4 row tiles of 128 (partition dim).
Local row tile r, local row t' -> global row 1024*r + m + 8*t'.
Row tile r attends to columns [0, 1024*(r+1)): col chunks c = 0..2r+1 of 512.
Chunks c = 2r, 2r+1 are the "diagonal" (mask halves 0/1); earlier chunks are
causally full. Each core does an identical 20-unit S/PV schedule.

Precision: scores must be ~fp32-accurate (softmax of std~1024 logits is
argmax-sensitive; bf16 flips ~1.5% of rows). Q and S matmuls run either in
float32r (full fp32 operands, fast PE mode) or bf16x2 (hi/lo split, 3-term).
PV and OV matmuls are bf16 (error ~2e-3, well under tolerance).
"""
import numpy as np
import ml_dtypes

import concourse.bass as bass
import concourse.tile as tile
from concourse import bacc, mybir
from concourse.bass_utils import run_bass_kernel_spmd
from concourse.masks import make_identity

F32 = mybir.dt.float32
BF16 = mybir.dt.bfloat16

N = 4096          # sequence length
D = 1024          # model dim
CORES = 8
ROWS = N // CORES  # 512 local rows per core
RT = ROWS // 128   # 4 row tiles
KC = D // 128      # 8 contraction chunks
MASK_VAL = -1e9

PRECISION = "bf16x2"   # "bf16x2" | "f32"

_BF = ml_dtypes.bfloat16


def _bf16(a):
    return np.asarray(a, dtype=_BF)


def _split_hi_lo(a):
    hi = _bf16(a)
    lo = _bf16(a.astype(np.float32) - hi.astype(np.float32))
    return hi, lo


def build_nc(precision=PRECISION):
    nc = bacc.Bacc("TRN2", target_bir_lowering=False, debug=False,
                   num_devices=CORES)

    def inp(name, shape, dt):
        return nc.dram_tensor(name, shape, dt, kind="ExternalInput").ap()

    if precision == "bf16x2":
        wqk_hi = inp("wqk_hi", [D, D], BF16)
        wqk_lo = inp("wqk_lo", [D, D], BF16)
        xqT_hi = inp("xqT_hi", [D, ROWS], BF16)
        xqT_lo = inp("xqT_lo", [D, ROWS], BF16)
        xt_hi = inp("xt_hi", [D, N], BF16)
        xt_lo = inp("xt_lo", [D, N], BF16)
    else:
        wqk_d = inp("wqk", [D, D], F32)
        xqT_d = inp("xqT", [D, ROWS], F32)
        xt_d = inp("xt", [D, N], F32)
    xv_d = inp("xv", [N, D], BF16)
    wov_d = inp("wov", [D, D], BF16)
    mask_d = inp("mask", [128, 1024], F32)
    out_d = nc.dram_tensor("out", [ROWS, D], F32, kind="ExternalOutput").ap()

    with tile.TileContext(nc) as tc:
        with (
            tc.tile_pool(name="sb", bufs=1) as sb,
            tc.tile_pool(name="sb2", bufs=2) as sb2,
            tc.tile_pool(name="xvp", bufs=4) as xvp,
            tc.tile_pool(name="ps_mm", bufs=2, space="PSUM") as ps_mm,
            tc.tile_pool(name="ps_att", bufs=2, space="PSUM") as ps_att,
            tc.tile_pool(name="ps_tp", bufs=2, space="PSUM") as ps_tp,
        ):
            ident = sb.tile([128, 128], BF16, tag="ident")
            make_identity(nc, ident[:])
            mask_sb = sb.tile([128, 1024], F32, tag="mask")
            nc.sync.dma_start(mask_sb[:], mask_d[:])

            # ---- load Q-side weights/inputs ----
            # wqk chunks: chunk k (contraction rows 128k..) at free offset 1024k
            if precision == "bf16x2":
                # tag-sharing: these Q-phase tiles are released after the Q
                # matmuls; s_all/p_all/pt_all then reuse the same slots.
                wqk_sb_h = sb.tile([128, KC * D], BF16, tag="wqk_s")
                wqk_sb_l = sb.tile([128, KC * D], BF16, tag="wqk_l_p")
                xqT_sb_h = sb.tile([128, KC * ROWS], BF16, tag="xq_pt")
                xqT_sb_l = sb.tile([128, KC * ROWS], BF16, tag="xq_l")
                for k in range(KC):
                    nc.sync.dma_start(wqk_sb_h[:, bass.ts(k, D)],
                                      wqk_hi[bass.ts(k, 128), :])
                    nc.sync.dma_start(wqk_sb_l[:, bass.ts(k, D)],
                                      wqk_lo[bass.ts(k, 128), :])
                    nc.sync.dma_start(xqT_sb_h[:, bass.ts(k, ROWS)],
                                      xqT_hi[bass.ts(k, 128), :])
                    nc.sync.dma_start(xqT_sb_l[:, bass.ts(k, ROWS)],
                                      xqT_lo[bass.ts(k, 128), :])
            else:
                wqk_sb = sb.tile([128, KC * D], F32, tag="wqk_s")
                xqT_sb = sb.tile([128, KC * ROWS], F32, tag="xq_p")
                for k in range(KC):
                    nc.sync.dma_start(wqk_sb[:, bass.ts(k, D)],
                                      wqk_d[bass.ts(k, 128), :])
                    nc.sync.dma_start(xqT_sb[:, bass.ts(k, ROWS)],
                                      xqT_d[bass.ts(k, 128), :])

            # ---- Q phase: QT[d', t] = sum_d wqk[d, d'] * xq[t, d] ----
            # QT stored as KC chunks [128 d'-tile, 512 t] side by side.
            if precision == "bf16x2":
                qt_h = sb.tile([128, KC * ROWS], BF16, tag="qt_h")
                qt_l = sb.tile([128, KC * ROWS], BF16, tag="qt_l")
            else:
                qt_sb = sb.tile([128, KC * ROWS], F32, tag="qt")

            for q in range(KC):
                acc = ps_mm.tile([128, ROWS], F32, tag="mm")
                if precision == "bf16x2":
                    terms = [(wqk_sb_h, xqT_sb_h), (wqk_sb_h, xqT_sb_l),
                             (wqk_sb_l, xqT_sb_h)]
                else:
                    terms = [(wqk_sb, xqT_sb)]
                n_mm = len(terms) * KC
                i = 0
                for lhs_t, rhs_t in terms:
                    for k in range(KC):
                        lhs = lhs_t[:, k * D + q * 128: k * D + (q + 1) * 128]
                        rhs = rhs_t[:, bass.ts(k, ROWS)]
                        nc.tensor.matmul(acc[:], lhs, rhs,
                                         start=(i == 0), stop=(i == n_mm - 1))
                        i += 1
                if precision == "bf16x2":
                    nc.vector.tensor_copy(qt_h[:, bass.ts(q, ROWS)], acc[:])
                    nc.vector.tensor_sub(qt_l[:, bass.ts(q, ROWS)], acc[:],
                                         qt_h[:, bass.ts(q, ROWS)])
                else:
                    nc.vector.tensor_copy(qt_sb[:, bass.ts(q, ROWS)], acc[:])

            # ---- S phase (col-chunk outer) + per-row-tile softmax/PV/OV ----
            s_off = [0, 1024, 3072, 6144]       # f32 elements into s_all
            s_len = [(2 * r + 2) * 512 for r in range(RT)]
            if precision == "bf16x2":
                s_all = sb.tile([128, 10240], F32, tag="wqk_s")
                p_all = sb.tile([128, 10240], BF16, tag="wqk_l_p")
                pt_all = sb.tile([128, 10240], BF16, tag="xq_pt")
            else:
                s_all = sb.tile([128, 10240], F32, tag="wqk_s")
                p_all = sb.tile([128, 10240], BF16, tag="xq_p")
                pt_all = sb.tile([128, 10240], BF16, tag="pt_all")

            negmax = sb.tile([128, RT], F32, tag="negmax")
            lsum = sb.tile([128, RT], F32, tag="lsum")
            linv = sb.tile([128, RT], F32, tag="linv")

            def s_chunk(r, c):
                """S[row tile r][col chunk c] -> s_all"""
                acc = ps_mm.tile([128, 512], F32, tag="mm")
                if precision == "bf16x2":
                    terms = [(qt_h, xt_h_sb), (qt_h, xt_l_sb), (qt_l, xt_h_sb)]
                else:
                    terms = [(qt_sb, xt_sb)]
                n_mm = len(terms) * KC
                i = 0
                for lhs_t, rhs_t in terms:
                    for k in range(KC):
                        lhs = lhs_t[:, k * ROWS + r * 128: k * ROWS + (r + 1) * 128]
                        rhs = rhs_t[:, bass.ts(k, 512)]
                        nc.tensor.matmul(acc[:], lhs, rhs,
                                         start=(i == 0), stop=(i == n_mm - 1))
                        i += 1
                dst = s_all[:, s_off[r] + c * 512: s_off[r] + (c + 1) * 512]
                if c == 2 * r:
                    nc.vector.tensor_add(dst, acc[:], mask_sb[:, 0:512])
                elif c == 2 * r + 1:
                    nc.vector.tensor_add(dst, acc[:], mask_sb[:, 512:1024])
                else:
                    nc.scalar.copy(dst, acc[:])

            def softmax_rt(r):
                s_r = s_all[:, s_off[r]: s_off[r] + s_len[r]]
                p_r = p_all[:, s_off[r]: s_off[r] + s_len[r]]
                nm = negmax[:, r: r + 1]
                nc.vector.tensor_reduce(out=nm, in_=s_r, op=mybir.AluOpType.max,
                                        axis=mybir.AxisListType.X, negate=True)
                nc.scalar.activation(p_r, s_r, mybir.ActivationFunctionType.Exp,
                                     bias=nm, scale=1.0,
                                     accum_out=lsum[:, r: r + 1])
                nc.vector.reciprocal(linv[:, r: r + 1], lsum[:, r: r + 1])
                # transpose P chunks of 128 cols, batch 4 per psum tile
                nchunks = s_len[r] // 128
                for g in range(nchunks // 4):
                    pt_ps = ps_tp.tile([128, 512], BF16, tag="tp")
                    for i in range(4):
                        jc = g * 4 + i
                        nc.tensor.matmul(
                            pt_ps[:, bass.ts(i, 128)],
                            p_all[:, s_off[r] + jc * 128: s_off[r] + (jc + 1) * 128],
                            ident[:], is_transpose=True,
                            start=(i == 0), stop=(i == 3))
                    nc.vector.tensor_copy(
                        pt_all[:, s_off[r] + g * 512: s_off[r] + (g + 1) * 512],
                        pt_ps[:])

            def pv_ov_rt(r):
                njc = 8 * (r + 1)
                att_ps = [ps_att.tile([128, 512], F32, tag="att",
                                      name=f"att_r{r}h{h}")
                          for h in range(2)]
                for jc in range(njc):
                    xv_c = xvp.tile([128, D], BF16, tag="xv")
                    nc.sync.dma_start(xv_c[:], xv_d[bass.ts(jc, 128), :])
                    lhs = pt_all[:, s_off[r] + jc * 128: s_off[r] + (jc + 1) * 128]
                    for h in range(2):
                        nc.tensor.matmul(att_ps[h][:], lhs,
                                         xv_c[:, bass.ts(h, 512)],
                                         start=(jc == 0), stop=(jc == njc - 1))
                # finalize: att = att_psum / L  (bf16), then transpose
                att_sb = sb2.tile([128, D], BF16, tag="att_sb")
                for h in range(2):
                    nc.scalar.mul(att_sb[:, bass.ts(h, 512)], att_ps[h][:],
                                  linv[:, r: r + 1])
                attT = sb2.tile([128, D], BF16, tag="attT")
                for g in range(2):
                    at_ps = ps_tp.tile([128, 512], BF16, tag="tp")
                    for i in range(4):
                        kk = g * 4 + i
                        nc.tensor.matmul(at_ps[:, bass.ts(i, 128)],
                                         att_sb[:, bass.ts(kk, 128)],
                                         ident[:], is_transpose=True,
                                         start=(i == 0), stop=(i == 3))
                    nc.vector.tensor_copy(attT[:, bass.ts(g, 512)], at_ps[:])
                # OV: out[t, :] = attT.T @ wov
                out_sb = sb2.tile([128, D], F32, tag="out_sb")
                for h in range(2):
                    acc = ps_mm.tile([128, 512], F32, tag="mm")
                    for k in range(KC):
                        nc.tensor.matmul(
                            acc[:], attT[:, bass.ts(k, 128)],
                            wov_sb[:, k * D + h * 512: k * D + (h + 1) * 512],
                            start=(k == 0), stop=(k == KC - 1))
                    nc.scalar.copy(out_sb[:, bass.ts(h, 512)], acc[:])
                nc.sync.dma_start(out_d[bass.ts(r, 128), :], out_sb[:])

            # wov load (chunk k at offset 1024k)
            wov_sb = sb.tile([128, KC * D], BF16, tag="wov")
            for k in range(KC):
                nc.sync.dma_start(wov_sb[:, bass.ts(k, D)],
                                  wov_d[bass.ts(k, 128), :])

            if precision == "bf16x2":
                xt_h_tiles, xt_l_tiles = [], []
            for c in range(2 * RT):
                # DMA xt col-chunk c: [D, 512] as KC chunks [128, 512]
                if precision == "bf16x2":
                    xt_h_sb = sb2.tile([128, KC * 512], BF16, tag="xt_h")
                    xt_l_sb = sb2.tile([128, KC * 512], BF16, tag="xt_l")
                    for k in range(KC):
                        nc.sync.dma_start(
                            xt_h_sb[:, bass.ts(k, 512)],
                            xt_hi[bass.ts(k, 128), bass.ts(c, 512)])
                        nc.sync.dma_start(
                            xt_l_sb[:, bass.ts(k, 512)],
                            xt_lo[bass.ts(k, 128), bass.ts(c, 512)])
                else:
                    xt_sb = sb2.tile([128, KC * 512], F32, tag="xt")
                    for k in range(KC):
                        nc.sync.dma_start(
                            xt_sb[:, bass.ts(k, 512)],
                            xt_d[bass.ts(k, 128), bass.ts(c, 512)])
                for r in range(c // 2, RT):
                    s_chunk(r, c)
                if c % 2 == 1:
                    r_done = (c - 1) // 2
                    softmax_rt(r_done)
                    pv_ov_rt(r_done)

    nc.compile()
    return nc


_NC_CACHE = {}


def _get_nc(precision):
    if precision not in _NC_CACHE:
        _NC_CACHE[precision] = build_nc(precision)
    return _NC_CACHE[precision]


def make_in_maps(x, wqk, wov, precision=PRECISION):
    x = np.ascontiguousarray(x, dtype=np.float32)
    wqk = np.ascontiguousarray(wqk, dtype=np.float32)
    wov = np.ascontiguousarray(wov, dtype=np.float32)

    xv = _bf16(x)
    wov_bf = _bf16(wov)
    xt = np.ascontiguousarray(x.T)

    shared = {"xv": xv, "wov": wov_bf}
    if precision == "bf16x2":
        wqk_hi, wqk_lo = _split_hi_lo(wqk)
        xt_hi, xt_lo = _split_hi_lo(xt)
        shared.update({"wqk_hi": wqk_hi, "wqk_lo": wqk_lo,
                       "xt_hi": xt_hi, "xt_lo": xt_lo})
    else:
        shared.update({"wqk": wqk, "xt": xt})

    in_maps = []
    t_idx = np.arange(128)
    c_idx = np.arange(1024)
    for m in range(CORES):
        xq = x[m::CORES]  # [512, 1024]
        xqT = np.ascontiguousarray(xq.T)
        mask = np.where(c_idx[None, :] <= m + 8 * t_idx[:, None],
                        0.0, MASK_VAL).astype(np.float32)
        im = dict(shared)
        im["mask"] = mask
        if precision == "bf16x2":
            h, l = _split_hi_lo(xqT)
            im["xqT_hi"] = h
            im["xqT_lo"] = l
        else:
            im["xqT"] = xqT
        in_maps.append(im)
    return in_maps


def kernel(x, wqk, wov, precision=PRECISION, _trace=False):
    nc = _get_nc(precision)
    in_maps = make_in_maps(x, wqk, wov, precision)
    res = run_bass_kernel_spmd(nc, in_maps, core_ids=list(range(CORES)),
                               trace=_trace)
    out = np.empty((N, D), dtype=np.float32)
    for m in range(CORES):
        out[m::CORES] = res.results[m]["out"]
    if _trace:
        kernel.last_results = res
    return out
